# revision 1
# baseline (speedup 1.0000x reference)
"""Trainium2 Bass kernel for CCPLoss:
out = sigmoid(mean(|maxpool35(min_c restored) - maxpool35(min_c target)|))

Inputs: restored, target: [16, 3, 512, 512] fp32.
Sharding: pure data parallel over batch; 2 images per core on 8 cores.
Per-core partial |diff| sums are reduced on host, then mean+sigmoid on host.

Per (restored, target) pair — both images ride in the same tiles (8 row
chunks: r in 0-3, t in 4-7) so every DVE op covers the pair at once:
 - SWDGE cast DMAs load fp32->bf16, channel-major.
 - channel-min: two bf16 tensor_tensor mins on DVE (2x mode).
 - separable 35x35 stride-1 max pool with zero padding (data >= 0, so 0
   behaves as -inf): shift-max doubling along the free dim, shifts
   {1,2,4,8,16,3} (subset sums cover 0..34), six 2x bf16 tensor_tensor max
   ops per axis, ping-pong buffers.
 - W pass in natural layout; PE transpose (identity matmul, 128x128 blocks,
   bf16 PSUM) + ACT copies into padded layout; H pass on transposed data.
Tail: diff on DVE, ACT Abs with fused accumulate (split in halves so ACT
overlaps DVE); host sums partials in float64, then mean + sigmoid.
"""

import sys

for _p in ("/opt/trn_rl_repo",):
    if _p not in sys.path:
        sys.path.insert(0, _p)

import numpy as np

import concourse.bass as bass
import concourse.mybir as mybir
from concourse import bacc, masks
from concourse.bass_utils import run_bass_kernel_spmd
from concourse.tile import TileContext

F32 = mybir.dt.float32
BF16 = mybir.dt.bfloat16
ALU = mybir.AluOpType

N_CORES = 8
B_FULL = 16
B_PER_CORE = B_FULL // N_CORES  # 2
C = 3
H = W = 512
K = 35
PAD = K // 2  # 17
NB = 546  # padded per-chunk length (512 + 2*17)
NCHUNK = 4  # 512 rows = 4 chunks of 128 partitions
FDP = NCHUNK * NB  # 2184
FD = NCHUNK * W  # 2048
SHIFTS = (1, 2, 4, 8, 16, 3)  # subset sums cover 0..34

_COMPILED = None


def _build_nc(reps=1, sim_safe=False):
    nc = bacc.Bacc("TRN2", detect_race_conditions=False)
    restored = nc.declare_dram_parameter(
        "restored", [B_PER_CORE, C, H, W], F32, isOutput=False
    )
    target = nc.declare_dram_parameter(
        "target", [B_PER_CORE, C, H, W], F32, isOutput=False
    )
    partial = nc.declare_dram_parameter("partial", [128, 2], F32, isOutput=True)

    with (
        TileContext(nc) as tc,
        tc.tile_pool(name="const", bufs=1) as cpool,
        tc.tile_pool(name="work", bufs=1) as pool,
        tc.tile_pool(name="psum", bufs=4, space="PSUM") as ppool,
    ):
        NG = 2 * NCHUNK  # pair-combined chunk count (r: 0-3, t: 4-7)
        GFDP = NG * NB   # 4368
        GFD = NG * W     # 4096

        def load_pair(b, rep):
            """Both images of pair b into one tile, channel-major:
            Xc[p, ch, img, c, w] so channel-min ops span the whole pair."""
            Xc = pool.tile([128, C * GFD], BF16, tag="Xc", bufs=2,
                           name=f"Xc_{b}_{rep}")
            Xc5 = Xc.rearrange("p (ch i c w) -> p ch i c w", ch=C, i=2, w=W)
            # channel-major issue order: min(ch0, ch1) only needs the first
            # four DMAs, so it starts while ch2 is still streaming
            for ch in range(C):
                for i, inp in enumerate((restored, target)):
                    src = inp[b, ch].rearrange("(c p) w -> p c w", p=128)
                    nc.gpsimd.dma_start(Xc5[:, ch, i], src)
            return Xc

        def maxpool_1d(Xp, out, who):
            """Sliding-window-35 max along the free dim of the padded
            [128, NG, NB] view Xp; writes [128, NG, W] into out."""
            A = pool.tile([128, GFDP], BF16, tag="A", bufs=2, name=f"A_{who}")
            Bt = pool.tile([128, GFDP], BF16, tag="B", bufs=2, name=f"B_{who}")
            A3 = A.rearrange("p (g n) -> p g n", n=NB)
            B3 = Bt.rearrange("p (g n) -> p g n", n=NB)
            bufs = [Xp, A3, B3, A3, B3, A3]
            cov = 1
            for j, s in enumerate(SHIFTS):
                src3 = bufs[j]
                dst3 = out if j == len(SHIFTS) - 1 else bufs[j + 1]
                span = W if j == len(SHIFTS) - 1 else NB - cov - s + 1
                nc.vector.tensor_tensor(
                    dst3[:, :, 0:span],
                    src3[:, :, 0:span],
                    src3[:, :, s : s + span],
                    ALU.max,
                )
                cov += s
            assert cov == K

        def w_phase(Xc, who, first):
            """channel-min + W-axis pool for a pair. Returns Rw [128, GFD]."""
            V = Xc.rearrange("p (ch m) -> p ch m", ch=C)
            nc.vector.tensor_tensor(V[:, 0], V[:, 0], V[:, 1], ALU.min)
            X = pool.tile([128, GFDP], BF16, tag="X", bufs=2, name=f"X_{who}")
            X3 = X.rearrange("p (g n) -> p g n", n=NB)
            if first:
                nc.vector.memset(X3[:, :, 0:PAD], 0.0)
                nc.vector.memset(X3[:, :, H + PAD : NB], 0.0)
            nc.vector.tensor_tensor(
                X3[:, :, PAD : PAD + W],
                V[:, 0].rearrange("p (g w) -> p g w", w=W),
                V[:, 2].rearrange("p (g w) -> p g w", w=W),
                ALU.min,
            )
            Rw = pool.tile([128, GFD], BF16, tag="Rw", bufs=2, name=f"Rw_{who}")
            maxpool_1d(X3, Rw.rearrange("p (g n) -> p g n", n=W), f"w{who}")
            return Rw

        def h_phase(Rw, who, first, ident):
            """PE transpose + H-axis pool for a pair. Returns RT [128, GFD]."""
            X2 = pool.tile([128, GFDP], BF16, tag="X2", bufs=2, name=f"X2_{who}")
            X23 = X2.rearrange("p (g n) -> p g n", n=NB)
            if first:
                nc.vector.memset(X23[:, :, 0:PAD], 0.0)
                nc.vector.memset(X23[:, :, H + PAD : NB], 0.0)
            for d in range(NCHUNK):
                ps = ppool.tile([128, 1024], BF16, tag="ps", name=f"ps_{who}_{d}")
                for i in range(2):
                    for c2 in range(NCHUNK):
                        nc.tensor.transpose(
                            ps[:, i * 512 + c2 * 128 : i * 512 + c2 * 128 + 128],
                            Rw[:, (i * NCHUNK + c2) * W + d * 128 :
                               (i * NCHUNK + c2) * W + d * 128 + 128],
                            ident[:],
                        )
                # one copy lands chunk d of both images: slots d and 4+d
                nc.scalar.copy(
                    X23[:, d :: NCHUNK, PAD : PAD + H],
                    ps.rearrange("p (i h) -> p i h", h=512),
                )
            RT = pool.tile([128, GFD], BF16, tag="RT", bufs=2, name=f"RT_{who}")
            maxpool_1d(X23, RT.rearrange("p (g n) -> p g n", n=H), f"h{who}")
            return RT

        smax = None
        for rep in range(reps):
            Xcs = [load_pair(b, rep) for b in range(B_PER_CORE)]
            if rep == 0:
                ident = cpool.tile([128, 128], BF16)
                masks.make_identity(nc, ident[:])
                smax = cpool.tile([128, 1], F32)
                nc.vector.memset(smax[:], 0.0)

            first = sim_safe or rep == 0
            Rws = [w_phase(Xcs[b], f"b{b}_{rep}", first) for b in range(B_PER_CORE)]
            accs = []
            for p in range(B_PER_CORE):
                RT = h_phase(Rws[p], f"h{p}_{rep}", first, ident)
                scr = pool.tile([128, FD], BF16, tag="scr", bufs=2, name=f"scr{p}_{rep}")
                sabs = pool.tile([128, FD], BF16, tag="sabs", bufs=2, name=f"sabs{p}_{rep}")
                hf = FD // 2
                for hx in range(2):
                    amax = pool.tile(
                        [128, 1], F32, tag="amax", bufs=4, name=f"am{p}_{rep}_{hx}"
                    )
                    sl = slice(hx * hf, (hx + 1) * hf)
                    sl_t = slice(FD + hx * hf, FD + (hx + 1) * hf)
                    nc.vector.tensor_tensor(
                        scr[:, sl], RT[:, sl], RT[:, sl_t], ALU.subtract
                    )
                    nc.scalar.activation(
                        sabs[:, sl], scr[:, sl],
                        mybir.ActivationFunctionType.Abs, accum_out=amax[:],
                    )
                    accs.append(amax)
            for amax in accs:
                nc.vector.tensor_tensor(smax[:], smax[:], amax[:], ALU.add)

        out2 = pool.tile([128, 2], F32)
        nc.vector.memset(out2[:, 1:2], 0.0)
        nc.vector.tensor_copy(out2[:, 0:1], smax[:])
        nc.sync.dma_start(partial[:], out2[:])

    nc.compile()
    return nc


def _get_compiled():
    global _COMPILED
    if _COMPILED is None:
        _COMPILED = _build_nc()
    return _COMPILED


def kernel(restored: np.ndarray, target: np.ndarray) -> np.ndarray:
    restored = np.ascontiguousarray(restored, dtype=np.float32)
    target = np.ascontiguousarray(target, dtype=np.float32)
    nc = _get_compiled()
    in_maps = []
    for i in range(N_CORES):
        sl = slice(i * B_PER_CORE, (i + 1) * B_PER_CORE)
        in_maps.append(
            {
                "restored": np.ascontiguousarray(restored[sl]),
                "target": np.ascontiguousarray(target[sl]),
            }
        )
    res = run_bass_kernel_spmd(nc, in_maps, list(range(N_CORES)))
    total = np.float64(0.0)
    for r in res.results:
        p = np.asarray(r["partial"], dtype=np.float64)
        total += p[:, 0].sum() - p[:, 1].sum()
    mean = total / float(B_FULL * H * W)
    out = 1.0 / (1.0 + np.exp(-mean))
    return np.asarray(out, dtype=np.float32)

